# revision 1
# baseline (speedup 1.0000x reference)
"""Trainium2 Bass kernel for XCA-style channel attention (Restormer MDTA block).

Pipeline per sample: 1x1 conv (PE matmul, fp16) -> 3x3 depthwise conv
(DVE/GPSIMD fused multiply-accumulate taps, fp16) -> per-head channel Gram via
DMA-xbar transposes + PE matmuls -> softmax (ACT/DVE) -> projection folded into
the attention-output matmul (PE).

Sharding: data-parallel over batch, 2 samples per core on 8 NeuronCores.
"""
import sys
sys.path.insert(0, '/opt/trn_rl_repo')
import numpy as np

import concourse.bass as bass
import concourse.tile as tile
from concourse import mybir, bacc
from concourse.bass_utils import run_bass_kernel_spmd

FP16 = mybir.dt.float16
F32 = mybir.dt.float32
ADD = mybir.AluOpType.add
MULT = mybir.AluOpType.mult
AF = mybir.ActivationFunctionType

B, C, H, W = 16, 192, 128, 128
HEADS, HD = 4, 48
N = H * W                 # 16384 spatial positions
NC_CORES = 8
SPC = B // NC_CORES       # samples per core = 2
G = 8                     # row groups per image
RG = H // G               # rows per group = 16
GC = RG * W               # out cols per group = 2048
C3 = 3 * C                # 576 qkv channels
# qkv channel chunks for the pointwise/DW stage
CHUNKS = [(0, 128), (128, 128), (256, 128), (384, 128), (512, 64)]
DW_ENGINE = ['vector', 'vector', 'vector', 'vector', 'vector']
TAPS = [(dy, dx) for dy in (-1, 0, 1) for dx in (-1, 0, 1)]
CENTER = 4
NEG_TAPS = [0, 3, 6, 2, 5, 8]  # dx==-1 taps then dx==+1 taps

_CACHE = {}
DEBUG = False
TRACE_SIM = False


def _build():
    nc = bacc.Bacc(None, target_bir_lowering=False)
    x_d = nc.declare_dram_parameter("x", [SPC, C, N], F32, isOutput=False)
    wqkvT_d = nc.declare_dram_parameter("wqkvT", [C, C3], FP16, isOutput=False)
    wdwp_d = nc.declare_dram_parameter("wdwp", [128, 45], F32, isOutput=False)
    wdwn_d = nc.declare_dram_parameter("wdwn", [128, 30], F32, isOutput=False)
    wprojT_d = nc.declare_dram_parameter("wprojT", [C, C], FP16, isOutput=False)
    tau_d = nc.declare_dram_parameter("tau", [96, 2], F32, isOutput=False)
    ident_d = nc.declare_dram_parameter("ident", [128, 128], FP16, isOutput=False)
    ones_d = nc.declare_dram_parameter("ones1", [1, 96], FP16, isOutput=False)
    emask_d = nc.declare_dram_parameter("emask", [96, 192], F32, isOutput=False)
    wdiag_d = nc.declare_dram_parameter("wdiag", [128, 45 * 128], FP16, isOutput=False)
    out_d = nc.declare_dram_parameter("out", [SPC, C, N], F32, isOutput=True)
    if DEBUG:
        dbg_qdw = nc.declare_dram_parameter("dbg_qdw", [128, GC], FP16, isOutput=True)
        dbg_pw = nc.declare_dram_parameter("dbg_pw", [128, 2434], FP16, isOutput=True)
        dbg_S = nc.declare_dram_parameter("dbg_S", [96, 192], F32, isOutput=True)
        dbg_Sraw = nc.declare_dram_parameter("dbg_Sraw", [96, 192], F32, isOutput=True)
        dbg_invn = nc.declare_dram_parameter("dbg_invn", [128, 3], F32, isOutput=True)
        dbg_qt = nc.declare_dram_parameter("dbg_qt", [128, RG, 96], FP16, isOutput=True)
        dbg_scq = nc.declare_dram_parameter("dbg_scq", [96, 2], F32, isOutput=True)
        dbg_ikrow = nc.declare_dram_parameter("dbg_ikrow", [1, 192], FP16, isOutput=True)
        dbg_ikrep = nc.declare_dram_parameter("dbg_ikrep", [96, 192], F32, isOutput=True)
        dbg_P = nc.declare_dram_parameter("dbg_P", [96, 192], FP16, isOutput=True)

    with tile.TileContext(nc, trace_sim=TRACE_SIM) as tc:
        with (
            tc.tile_pool(name="const", bufs=1) as cpool,
            tc.tile_pool(name="x16", bufs=3) as x16pool,
            tc.tile_pool(name="pw", bufs=6) as pwpool,
            tc.tile_pool(name="dwqk", bufs=5) as dwpool,
            tc.tile_pool(name="qt", bufs=3) as qtpool,
            tc.tile_pool(name="vdw", bufs=8) as vpool,
            tc.tile_pool(name="small", bufs=2) as spool,
            tc.tile_pool(name="sq", bufs=1) as sqpool,
            tc.tile_pool(name="sh", bufs=1) as shpool,
            tc.tile_pool(name="stage", bufs=3) as stpool,
            tc.tile_pool(name="pps", bufs=3, space="PSUM") as ppsum,
            tc.tile_pool(name="sps", bufs=1, space="PSUM") as spsum,
            tc.tile_pool(name="fps", bufs=1, space="PSUM") as fpsum,
            tc.tile_pool(name="wps", bufs=1, space="PSUM") as wpsum,
        ):
            # ---- constants ----
            wq_a = cpool.tile([96, C3], FP16)
            nc.sync.dma_start(wq_a[:], wqkvT_d[0:96, :])
            wq_b = cpool.tile([96, C3], FP16)
            nc.sync.dma_start(wq_b[:], wqkvT_d[96:192, :])
            wdwp0 = cpool.tile([128, 45], F32)
            nc.sync.dma_start(wdwp0[:], wdwp_d[:])
            wdwn0 = cpool.tile([128, 30], F32)
            nc.sync.dma_start(wdwn0[:], wdwn_d[:])
            # copy through DVE so TensorScalarPtr ops never wait on the DMA
            wdwp = cpool.tile([128, 45], F32)
            nc.vector.tensor_copy(wdwp[:], wdwp0[:])
            wdwn = cpool.tile([128, 30], F32)
            nc.vector.tensor_copy(wdwn[:], wdwn0[:])
            wproj_h = []
            for h in range(HEADS):
                t = cpool.tile([HD, C], FP16, tag=f"wproj{h}", name=f"wproj{h}")
                nc.sync.dma_start(t[:], wprojT_d[HD * h:HD * (h + 1), :])
                wproj_h.append(t)
            tau_sb = cpool.tile([96, 2], F32)
            nc.sync.dma_start(tau_sb[:], tau_d[:])
            ident = cpool.tile([128, 128], FP16)
            nc.sync.dma_start(ident[:], ident_d[:])
            ones1 = cpool.tile([1, 96], FP16)
            nc.sync.dma_start(ones1[:], ones_d[:])
            emask = cpool.tile([96, 192], F32)
            nc.sync.dma_start(emask[:], emask_d[:])
            wdiag = cpool.tile([128, 45 * 128], FP16)
            nc.sync.dma_start(wdiag[:], wdiag_d[:])

            for s in range(SPC):
                vtiles = {}
                S01_ps = spsum.tile([96, 96], F32, tag="S01", name=f"S01_{s}")
                S23_ps = spsum.tile([96, 96], F32, tag="S23", name=f"S23_{s}")
                npart = [spool.tile([128, G], F32, tag=f"np{i}", name=f"np{i}_{s}") for i in range(3)]
                qt_tiles = {}

                for g in range(G):
                    lo = 1 if g == 0 else 0
                    hi = 17 if g == G - 1 else 18
                    r0 = g * RG - 1          # image row of slot 0
                    ncols = (hi - lo) * W

                    # ---- load x rows [r0+lo, r0+hi): gpsimd DMA casts f32->fp16 ----
                    x16a = x16pool.tile([96, 18 * W], FP16, tag="x16")
                    x16b = x16pool.tile([96, 18 * W], FP16, tag="x16")
                    nc.gpsimd.dma_start(
                        x16a[:, lo * W:hi * W],
                        x_d[s, 0:96, (r0 + lo) * W:(r0 + hi) * W])
                    nc.gpsimd.dma_start(
                        x16b[:, lo * W:hi * W],
                        x_d[s, 96:192, (r0 + lo) * W:(r0 + hi) * W])

                    # ---- pointwise conv + depthwise conv per qkv chunk ----
                    accs = {}
                    for oc in (3, 0, 4, 1, 2):
                        cb, cw = CHUNKS[oc]
                        pw = pwpool.tile([128, 18 * W + 2 + W], FP16, tag="pw")
                        # pad elems flat 0 / 2305 (read by extreme shifts)
                        nc.gpsimd.memset(pw[:cw, 0:1], 0.0)
                        nc.gpsimd.memset(pw[:cw, 2305:2306], 0.0)
                        if g == 0:
                            nc.gpsimd.memset(pw[:cw, 1:1 + W], 0.0)
                        if g == G - 1:
                            nc.gpsimd.memset(pw[:cw, 1 + 17 * W:2305], 0.0)
                        col = lo * W
                        while col < hi * W:
                            nsub = min(512, hi * W - col)
                            ps = ppsum.tile([128, 512], F32, tag="pwps")
                            nc.tensor.matmul(ps[:cw, :nsub],
                                             wq_a[:, cb:cb + cw],
                                             x16a[:, col:col + nsub],
                                             start=True, stop=False)
                            nc.tensor.matmul(ps[:cw, :nsub],
                                             wq_b[:, cb:cb + cw],
                                             x16b[:, col:col + nsub],
                                             start=False, stop=True)
                            nc.scalar.copy(pw[:cw, 1 + col:1 + col + nsub],
                                           ps[:cw, :nsub])
                            col += nsub

                        # ---- depthwise 3x3: 9 MAC taps + 6 edge fixes ----
                        use_pe = oc in (0, 1) or (oc == 2 and g % 4 != 3)
                        eng = nc.vector
                        wp = wdwp
                        wn = wdwn
                        if oc == 3:
                            vga = vpool.tile([128, GC], FP16, tag="vdwa",
                                             name=f"vga_{s}_{g}")
                            vtiles.setdefault(g, {})[3] = vga
                            acc = vga[:, :]
                        elif oc == 4:
                            vgb = vpool.tile([64, GC], FP16, tag="vdwb",
                                             name=f"vgb_{s}_{g}")
                            vtiles.setdefault(g, {})[4] = vgb
                            acc = vgb[:, :]
                        else:
                            at = dwpool.tile([128, GC], FP16, tag="dw")
                            accs[oc] = at
                            acc = at[:cw, :]
                        if use_pe:
                            # depthwise as 9 PSUM-accumulated diag matmuls
                            for sub in range(4):
                                dps = ppsum.tile([128, 512], F32, tag="pwps", name=f"dps_{s}_{g}_{oc}_{sub}")
                                for t, (dy, dx) in enumerate(TAPS):
                                    d = dy * W + dx
                                    c0 = (9 * oc + t) * 128
                                    nc.tensor.matmul(
                                        dps[:cw, :],
                                        wdiag[:cw, c0:c0 + cw],
                                        pw[:cw, 129 + d + sub * 512:
                                           129 + d + sub * 512 + 512],
                                        start=(t == 0), stop=(t == 8))
                                nc.scalar.copy(acc[:, sub * 512:(sub + 1) * 512]
                                               if oc < 3 else
                                               acc[:, sub * 512:(sub + 1) * 512],
                                               dps[:cw, :])
                        else:
                            # dual-parity buffers keep every tensor_scalar read
                            # 4B-aligned: 4x-mode TS product + 2x-mode TT add
                            pwsh = shpool.tile([128, 18 * W + 2 + W], FP16,
                                               tag="pwsh",
                                               name=f"pwsh_{s}_{g}_{oc}")
                            nc.vector.tensor_copy(pwsh[:cw, 0:2433],
                                                  pw[:cw, 1:2434])
                            nc.vector.tensor_scalar_mul(
                                acc, pwsh[:cw, 128:128 + GC],
                                wp[:cw, 9 * oc + CENTER:9 * oc + CENTER + 1])
                            for t, (dy, dx) in enumerate(TAPS):
                                if t == CENTER:
                                    continue
                                d = dy * W + dx
                                tmp = shpool.tile([128, GC], FP16, tag="ttmp",
                                                  name=f"ttmp_{s}_{g}_{oc}_{t}")
                                if dx == 0:
                                    src_ap = pwsh[:cw, 128 + d:128 + d + GC]
                                else:
                                    src_ap = pw[:cw, 129 + d:129 + d + GC]
                                nc.vector.tensor_scalar_mul(
                                    tmp[:cw, :], src_ap,
                                    wp[:cw, 9 * oc + t:9 * oc + t + 1])
                                nc.vector.tensor_add(acc, acc, tmp[:cw, :])
                        # wrap-around corrections on columns w=0 and w=127
                        acc_v = acc.rearrange("p (r w) -> p r w", w=W)
                        for j, t in enumerate(NEG_TAPS):
                            dy = TAPS[t][0]
                            if TAPS[t][1] == -1:
                                src0 = W * (1 + dy)
                                o_ap = acc_v[:, :, 0:1]
                            else:
                                src0 = 257 + W * dy
                                o_ap = acc_v[:, :, W - 1:W]
                            i_ap = pw[:cw, src0:src0 + GC].rearrange(
                                "p (r w) -> p r w", w=W)[:, :, 0:1]
                            eng.scalar_tensor_tensor(
                                o_ap, i_ap,
                                wn[:cw, 6 * oc + j:6 * oc + j + 1],
                                o_ap, op0=MULT, op1=ADD)

                    if DEBUG and s == 0 and g == 0:
                        nc.sync.dma_start(dbg_qdw[:], accs[0][:])
                        nc.sync.dma_start(dbg_pw[:], pw[:])
                    # ---- norms (sum of squares) for q,k chunks on ACT ----
                    sqd = sqpool.tile([128, GC], FP16, tag="sqd")
                    for oc in range(3):
                        cw = CHUNKS[oc][1]
                        nc.scalar.activation(
                            sqd[:cw, :], accs[oc][:cw, :], AF.Square,
                            accum_out=npart[oc][:cw, g:g + 1])

                    # ---- transposes for the Gram (DMA xbar) ----
                    qt = {k: qtpool.tile([128, RG, 96], FP16, tag=k, name=f"{k}_{s}_{g}")
                          for k in ("q01", "q23", "k01", "k23")}
                    qt_tiles[g] = qt
                    nc.sync.dma_start_transpose(qt["q01"][:, :, 0:96],
                                                accs[0][0:96, :])
                    nc.sync.dma_start_transpose(qt["q23"][:, :, 0:32],
                                                accs[0][96:128, :])
                    nc.sync.dma_start_transpose(qt["q23"][:, :, 32:96],
                                                accs[1][0:64, :])
                    nc.sync.dma_start_transpose(qt["k01"][:, :, 0:64],
                                                accs[1][64:128, :])
                    nc.sync.dma_start_transpose(qt["k01"][:, :, 64:96],
                                                accs[2][0:32, :])
                    nc.sync.dma_start_transpose(qt["k23"][:, :, 0:32],
                                                accs[2][32:64, :])
                    nc.sync.dma_start_transpose(qt["k23"][:, :, 32:96],
                                                accs[2][64:128, :])

                    if DEBUG and s == 0 and g == 0:
                        nc.sync.dma_start(dbg_qt[:], qt["q01"][:])
                    # ---- Gram accumulation: S[c,d] += sum_n q[c,n] k[d,n] ----
                    for r in range(RG):
                        first = (g == 0 and r == 0)
                        last = (g == G - 1 and r == RG - 1)
                        nc.tensor.matmul(S01_ps[:], qt["q01"][:, r, :],
                                         qt["k01"][:, r, :],
                                         start=first, stop=last)
                        nc.tensor.matmul(S23_ps[:], qt["q23"][:, r, :],
                                         qt["k23"][:, r, :],
                                         start=first, stop=last)

                # ======== phase B: norms -> softmax -> Wp2 -> output ========
                nsum = spool.tile([128, 3], F32, tag="nsum")
                inv_n = spool.tile([128, 3], F32, tag="invn")
                for oc in range(3):
                    nc.vector.tensor_reduce(nsum[:, oc:oc + 1], npart[oc][:],
                                            axis=mybir.AxisListType.X, op=ADD)
                nc.scalar.activation(nsum[:], nsum[:], AF.Sqrt)
                nc.vector.tensor_scalar_max(nsum[:], nsum[:], 1e-12)
                nc.vector.reciprocal(inv_n[:], nsum[:])

                # gather per-pair inverse-norm vectors ([96,1] at base 0)
                iq01 = spool.tile([96, 1], F32, tag="iq01")
                nc.sync.dma_start(iq01[:], inv_n[0:96, 0:1])
                iq23 = spool.tile([96, 1], F32, tag="iq23")
                nc.sync.dma_start(iq23[0:32, :], inv_n[96:128, 0:1])
                nc.sync.dma_start(iq23[32:96, :], inv_n[0:64, 1:2])
                ik01 = spool.tile([96, 1], F32, tag="ik01")
                nc.sync.dma_start(ik01[0:64, :], inv_n[64:128, 1:2])
                nc.sync.dma_start(ik01[64:96, :], inv_n[0:32, 2:3])
                ik23 = spool.tile([96, 1], F32, tag="ik23")
                nc.sync.dma_start(ik23[:], inv_n[32:128, 2:3])
                # row scales tau/||q||
                sc_q = spool.tile([96, 2], F32, tag="scq")
                nc.vector.tensor_mul(sc_q[:, 0:1], iq01[:], tau_sb[:, 0:1])
                nc.vector.tensor_mul(sc_q[:, 1:2], iq23[:], tau_sb[:, 1:2])
                # replicated col scales 1/||k||: transpose cols via PE
                ikh = spool.tile([96, 2], FP16, tag="ikh")
                nc.vector.tensor_copy(ikh[:, 0:1], ik01[:])
                nc.vector.tensor_copy(ikh[:, 1:2], ik23[:])
                ikrow = spool.tile([1, 192], FP16, tag="ikrow")
                for p in range(2):
                    ikps = wpsum.tile([1, 96], FP16, tag="trps",
                                      name=f"ikps{p}_{s}")
                    nc.tensor.transpose(ikps[:], ikh[:, p:p + 1],
                                        ident[0:96, 0:96])
                    nc.scalar.copy(ikrow[:, 96 * p:96 * (p + 1)], ikps[:])
                ikrep = wpsum.tile([96, 192], F32, tag="wp2ps", name=f"ikrep_{s}")
                nc.tensor.matmul(ikrep[:], ones1[:], ikrow[:],
                                 start=True, stop=True)

                if DEBUG and s == 0:
                    sr = spool.tile([96, 192], F32, tag="dbgsr")
                    nc.scalar.copy(sr[:, 0:96], S01_ps[:])
                    nc.scalar.copy(sr[:, 96:192], S23_ps[:])
                    nc.sync.dma_start(dbg_Sraw[:], sr[:])
                    nc.sync.dma_start(dbg_invn[:], inv_n[:])
                if DEBUG and s == 0:
                    nc.sync.dma_start(dbg_scq[:], sc_q[:])
                    nc.sync.dma_start(dbg_ikrow[:], ikrow[:])
                    ikr = spool.tile([96, 192], F32, tag="dbgikr")
                    nc.vector.tensor_copy(ikr[:], ikrep[:])
                    nc.sync.dma_start(dbg_ikrep[:], ikr[:])
                # logits = S * (tau/||q||_row) * (1/||k||_col)
                Ssb = spool.tile([96, 192], F32, tag="Ssb")
                nc.scalar.activation(Ssb[:, 0:96], S01_ps[:], AF.Copy,
                                     scale=sc_q[:, 0:1])
                nc.scalar.activation(Ssb[:, 96:192], S23_ps[:], AF.Copy,
                                     scale=sc_q[:, 1:2])
                nc.vector.tensor_mul(Ssb[:], Ssb[:], ikrep[:])

                if DEBUG and s == 0:
                    nc.sync.dma_start(dbg_S[:], Ssb[:])
                # softmax over each head's 48-col block
                rmax = spool.tile([96, 2], F32, tag="rmax")
                nrmax = spool.tile([96, 2], F32, tag="nrmax")
                for p in range(2):
                    nc.vector.tensor_reduce(rmax[:, p:p + 1],
                                            Ssb[:, 96 * p:96 * (p + 1)],
                                            axis=mybir.AxisListType.X,
                                            op=mybir.AluOpType.max)
                nc.vector.tensor_scalar_mul(nrmax[:], rmax[:], -1.0)
                Esb = spool.tile([96, 192], F32, tag="Esb")
                for p in range(2):
                    nc.scalar.activation(Esb[:, 96 * p:96 * (p + 1)],
                                         Ssb[:, 96 * p:96 * (p + 1)], AF.Exp,
                                         bias=nrmax[:, p:p + 1], scale=1.0)
                nc.vector.tensor_mul(Esb[:], Esb[:], emask[:])
                rsum = spool.tile([96, 2], F32, tag="rsum")
                rsinv = spool.tile([96, 2], F32, tag="rsinv")
                for p in range(2):
                    nc.vector.tensor_reduce(
                        rsum[:, p:p + 1], Esb[:, 96 * p:96 * (p + 1)],
                        axis=mybir.AxisListType.X, op=ADD)
                nc.vector.reciprocal(rsinv[:], rsum[:])
                Psb = spool.tile([96, 192], FP16, tag="Psb")
                for p in range(2):
                    nc.scalar.activation(Psb[:, 96 * p:96 * (p + 1)],
                                         Esb[:, 96 * p:96 * (p + 1)],
                                         AF.Copy, scale=rsinv[:, p:p + 1])
                if DEBUG and s == 0:
                    nc.sync.dma_start(dbg_P[:], Psb[:])
                # per-head P at partition base 0 (matmul rhs)
                P_h = [spool.tile([HD, HD], FP16, tag=f"P{h}", name=f"P{h}_{s}") for h in range(4)]
                nc.sync.dma_start(P_h[0][:], Psb[0:48, 0:48])
                nc.sync.dma_start(P_h[1][:], Psb[48:96, 48:96])
                nc.sync.dma_start(P_h[2][:], Psb[0:48, 96:144])
                nc.sync.dma_start(P_h[3][:], Psb[48:96, 144:192])

                # Wp2[o, 48h+d] = sum_c wproj[o, 48h+c] P_h[c, d]
                wp2 = [spool.tile([96, 192], FP16, tag=f"wp2{i}", name=f"wp2{i}_{s}") for i in range(2)]
                for i in range(2):
                    wps = wpsum.tile([96, 192], F32, tag="wp2ps")
                    for h in range(4):
                        nc.tensor.matmul(wps[:, 48 * h:48 * (h + 1)],
                                         wproj_h[h][:, 96 * i:96 * (i + 1)],
                                         P_h[h][:], start=True, stop=True)
                    nc.scalar.copy(wp2[i][:], wps[:])
                # transpose -> wp2T_a [128(d), 192(o)], wp2T_b [64(d), 192(o)]
                wp2T_a = spool.tile([128, 192], FP16, tag="wp2Ta")
                wp2T_b = spool.tile([64, 192], FP16, tag="wp2Tb")
                for i in range(2):
                    tps = wpsum.tile([128, 96], FP16, tag="trps")
                    nc.tensor.transpose(tps[:, :], wp2[i][:, 0:128], ident[0:96, 0:96])
                    nc.scalar.copy(wp2T_a[:, 96 * i:96 * (i + 1)], tps[:, :])
                    tps2 = wpsum.tile([64, 96], FP16, tag="trps")
                    nc.tensor.transpose(tps2[:, :], wp2[i][:, 128:192],
                                        ident[0:96, 0:96])
                    nc.scalar.copy(wp2T_b[:, 96 * i:96 * (i + 1)], tps2[:, :])

                # ---- final: out[o, n] = sum_d Wp2T[d, o] vdw[d, n] ----
                for g2 in range(G):
                    for i in range(2):          # output channel 96-chunks
                        for j in range(4):      # 512-col subtiles of the group
                            col = g2 * GC + j * 512
                            if (g2 * 8 + i * 4 + j) % 2 == 0:
                                fps = fpsum.tile([96, 512], F32, tag="fps",
                                                 name=f"f_{s}_{g2}_{i}_{j}")
                            else:
                                fps = wpsum.tile([96, 512], F32, tag="wp2ps",
                                                 name=f"f_{s}_{g2}_{i}_{j}")
                            vcol = j * 512
                            nc.tensor.matmul(fps[:], wp2T_a[:, 96 * i:96 * (i + 1)],
                                             vtiles[g2][3][:, vcol:vcol + 512],
                                             start=True, stop=False)
                            nc.tensor.matmul(fps[:], wp2T_b[:, 96 * i:96 * (i + 1)],
                                             vtiles[g2][4][:, vcol:vcol + 512],
                                             start=False, stop=True)
                            st = stpool.tile([96, 512], F32, tag="st")
                            nc.scalar.copy(st[:], fps[:])
                            nc.sync.dma_start(
                                out_d[s, 96 * i:96 * (i + 1), col:col + 512],
                                st[:])
    nc.finalize()
    return nc


def _prep_consts(w_qkv, w_dw, w_proj, temperature):
    wqkvT = np.ascontiguousarray(w_qkv[:, :, 0, 0].T).astype(np.float16)
    wdwp = np.zeros((128, 45), np.float32)
    wdwn = np.zeros((128, 30), np.float32)
    taps = w_dw[:, 0, :, :].reshape(C3, 9)  # tap t = (dy+1)*3+(dx+1)
    for oc, (cb, cw) in enumerate(CHUNKS):
        wdwp[:cw, 9 * oc:9 * oc + 9] = taps[cb:cb + cw, :]
        for j, t in enumerate(NEG_TAPS):
            wdwn[:cw, 6 * oc + j] = -taps[cb:cb + cw, t]
    wprojT = np.ascontiguousarray(w_proj[:, :, 0, 0].T).astype(np.float16)
    wdiag = np.zeros((128, 45 * 128), np.float16)
    for oc, (cb, cw) in enumerate(CHUNKS):
        for t in range(9):
            c0 = (9 * oc + t) * 128
            wdiag[np.arange(cw), c0 + np.arange(cw)] = taps[cb:cb + cw, t]
    tau = np.zeros((96, 2), np.float32)
    tf = temperature.reshape(4)
    tau[0:48, 0] = tf[0]; tau[48:96, 0] = tf[1]
    tau[0:48, 1] = tf[2]; tau[48:96, 1] = tf[3]
    ident = np.eye(128, dtype=np.float16)
    return dict(wqkvT=wqkvT, wdwp=wdwp, wdwn=wdwn, wprojT=wprojT,
                tau=tau, ident=ident, wdiag=wdiag,
                ones1=np.ones((1, 96), np.float16),
                emask=np.tile(np.kron(np.eye(2, dtype=np.float32),
                                      np.ones((48, 48), np.float32)), (1, 2)))


def kernel(x, w_qkv, w_dw, w_proj, temperature, _trace=False):
    x = np.asarray(x, np.float32)
    consts = _prep_consts(np.asarray(w_qkv, np.float32),
                          np.asarray(w_dw, np.float32),
                          np.asarray(w_proj, np.float32),
                          np.asarray(temperature, np.float32))
    if "nc" not in _CACHE:
        _CACHE["nc"] = _build()
    nc = _CACHE["nc"]
    xr = x.reshape(B, C, N)
    in_maps = [dict(x=np.ascontiguousarray(xr[SPC * i:SPC * (i + 1)]), **consts)
               for i in range(NC_CORES)]
    res = run_bass_kernel_spmd(nc, in_maps, core_ids=list(range(NC_CORES)),
                               trace=_trace)
    out = np.concatenate([res.results[i]["out"] for i in range(NC_CORES)], axis=0)
    if _trace:
        kernel.last_exec_time_ns = res.exec_time_ns
        kernel.last_profile = res.profile_json
    return out.reshape(B, C, H, W).astype(np.float32)

